# revision 6
# baseline (speedup 1.0000x reference)
"""Trainium2 Bass kernel for nn_CrossAttention (B=32, TF=2048, TP=256,
FRAME=768, PHN=512, ATT=512), data-parallel over batch on 8 NeuronCores.

Math per batch element (matches the jax reference):
    q  = frame @ Wq + bq                 [TF, A]
    k  = phn @ Wk + bk                   [TP, A]
    energy = q @ k.T + (1-mask)*(-1000)  [TF, TP]   (returned)
    att = softmax(energy, -1)
    out = LN(concat[att @ k, q]) * gamma + beta     (returned)

Device-side decomposition (avoids materializing q.T):
    kT = contraction of Wk with phnT     [A, TP]
    W2 = Wq @ kT                         [F, TP]
    bias_row = bq @ kT + maskbias        [TP]
    energy = frame @ W2 + ones (x) bias_row   (rank-1 matmul adds row bias)
Matmuls run as float32r (full-rate PE mode for N>=256); the BIR verifier
requires fp32r operands to be produced (rounded) by their writer, so every
matmul-operand tile is dtype float32r written by a DVE/ACT op. PE transposes
stay exact fp32.
"""

import numpy as np

import concourse.bass as bass
import concourse.tile as tile
from concourse import mybir
from concourse.bass_utils import run_bass_kernel_spmd
from concourse.masks import make_identity
from concourse.vector_clock import ScopedClock, VectorClock

F32 = mybir.dt.float32
F32R = mybir.dt.float32r
I32 = mybir.dt.int32
AF = mybir.ActivationFunctionType
AX = mybir.AxisListType
ALU = mybir.AluOpType

B, TF, TP = 32, 2048, 256
FD, PD, AD = 768, 512, 512
N_CORES = 8
BPC = B // N_CORES          # batch elements per core
NTC = TF // 512             # 512-row t-chunks per batch element
NF = FD // 128              # 6 f-tiles
NA = AD // 128              # 4 a-tiles
NP = TP // 128              # 2 p-tiles


def _patch_drain():
    """The packaged walrus rejects the TileContext tail Drain when it
    carries >1 sem wait; absorb the waits on single-wait SP nops first."""
    if getattr(tile.TileContext, "_drain_patched", False):
        return

    def _drain_and_barrier(self, tick_clock, wait_clock):
        vec = tick_clock.global_clock
        n = len(vec)
        for proc in range(n):
            tck = vec[proc]
            if tck <= 0:
                continue
            req = VectorClock([0] * n)
            req.require_at_least(proc, tck)
            nop = self.nc.sync.nop(nofuse=True, hint="drain_split_wait")
            wait_clock.add_sem_waits(nop.ins, ScopedClock({None: req}))
        self.nc.sync.drain()
        self.nc.all_engine_barrier()
        assert self.sems is not None
        popped = self.nc._tile_sem_poison_stack.pop()
        assert popped is self._sem_poison
        self.nc.clear_and_free_semaphores(list(self.sems.allocated().values()))
        self.nc.all_engine_barrier()

    tile.TileContext._drain_and_barrier = _drain_and_barrier
    tile.TileContext._drain_patched = True


_patch_drain()


def _split_excess_waits(nc, max_waits=1):
    """The packaged walrus rejects instructions carrying more than one sem
    wait; hoist extras onto same-engine NoOps placed just before them."""
    n_new = 0
    for f in nc.m.functions:
        for bb in f.blocks:
            insts = bb.instructions
            out = []
            changed = False
            for inst in insts:
                si = inst.sync_info
                if si is not None and len(si.on_wait) > max_waits:
                    waits = list(si.on_wait)
                    for w in waits[:-max_waits]:
                        nop = mybir.InstNoOp(
                            name=f"{inst.name}-wsplit{n_new}", ins=[], outs=[])
                        nop.engine = inst.engine
                        nop.sync_info = mybir.SyncInfo(
                            on_wait=[w], on_update=[])
                        out.append(nop)
                        n_new += 1
                    si.on_wait = waits[-max_waits:]
                    changed = True
                out.append(inst)
            if changed:
                bb.instructions = out
    return n_new


def build_program():
    nc = bass.Bass("TRN2", target_bir_lowering=False)

    frame_d = nc.dram_tensor("frame", [BPC, TF, FD], F32, kind="ExternalInput")
    phn_d = nc.dram_tensor("phn", [BPC, TP, PD], F32, kind="ExternalInput")
    mask_d = nc.dram_tensor("mask", [BPC, TP], I32, kind="ExternalInput")
    wq_d = nc.dram_tensor("wq", [FD, AD], F32, kind="ExternalInput")
    bq_d = nc.dram_tensor("bq", [AD], F32, kind="ExternalInput")
    wk_d = nc.dram_tensor("wk", [PD, AD], F32, kind="ExternalInput")
    bk_d = nc.dram_tensor("bk", [AD], F32, kind="ExternalInput")
    attout_d = nc.dram_tensor("attout", [BPC, TF, 2 * AD], F32,
                              kind="ExternalOutput")
    energy_d = nc.dram_tensor("energy", [BPC, TF, TP], F32,
                              kind="ExternalOutput")

    with (
        tile.TileContext(nc) as tc,
        tc.tile_pool(name="consts", bufs=1) as consts,
        tc.tile_pool(name="batchp", bufs=2) as batchp,
        tc.tile_pool(name="chunkp", bufs=2) as chunkp,
        tc.tile_pool(name="psA", bufs=2, space="PSUM") as psA,   # frameT
        tc.tile_pool(name="psB", bufs=2, space="PSUM") as psB,   # q / k
        tc.tile_pool(name="psC", bufs=2, space="PSUM") as psC,   # energy/W2
        tc.tile_pool(name="psD", bufs=1, space="PSUM") as psD,   # attT/phnT
        tc.tile_pool(name="psE", bufs=1, space="PSUM") as psE,   # attout/kT
    ):
        # ---- constants --------------------------------------------------
        ident = consts.tile([128, 128], F32, tag="ident")
        make_identity(nc, ident)
        eps_col = consts.tile([128, 1], F32, tag="eps_col")
        nc.vector.memset(eps_col, 1e-5)
        ones_f = consts.tile([1, 128], F32, tag="ones_f")
        nc.vector.memset(ones_f, 1.0)
        ones_col = consts.tile([1, 128], F32R, tag="ones_col")
        nc.vector.tensor_copy(out=ones_col, in_=ones_f)

        # fp32 staging loads (recycled chunk-pool slots), fp32r converts
        wq_tmp = chunkp.tile([128, NF * AD], F32, tag="frame")
        wq_tmp_v = wq_tmp.rearrange("p (n a) -> p n a", n=NF)
        nc.sync.dma_start(
            out=wq_tmp_v,
            in_=wq_d[:, :].rearrange("(n p) a -> p n a", p=128))
        wq_sb = consts.tile([128, NF, AD], F32R, tag="wq")   # [f%128, f//128, a]
        nc.scalar.activation(
            out=wq_sb.rearrange("p n a -> p (n a)"), in_=wq_tmp, func=AF.Copy)

        wk_tmp = chunkp.tile([128, NF * AD], F32, tag="frame")
        nc.sync.dma_start(
            out=wk_tmp[:, :NA * AD].rearrange("p (n a) -> p n a", n=NA),
            in_=wk_d[:, :].rearrange("(n p) a -> p n a", p=128))
        wk_sb = consts.tile([128, NA, AD], F32R, tag="wk")
        nc.scalar.activation(
            out=wk_sb.rearrange("p n a -> p (n a)"), in_=wk_tmp[:, :NA * AD],
            func=AF.Copy)

        bq_tmp = consts.tile([1, AD], F32, tag="bq_tmp")
        nc.sync.dma_start(out=bq_tmp, in_=bq_d[:].unsqueeze(0))
        bq_row = consts.tile([1, AD], F32R, tag="bq_row")
        nc.vector.tensor_copy(out=bq_row, in_=bq_tmp)
        bk_tmp = consts.tile([1, AD], F32, tag="bk_tmp")
        nc.sync.dma_start(out=bk_tmp, in_=bk_d[:].unsqueeze(0))
        bk_row = consts.tile([1, AD], F32R, tag="bk_row")
        nc.vector.tensor_copy(out=bk_row, in_=bk_tmp)
        bqc_tmp = consts.tile([128, NA], F32, tag="bqc_tmp")
        nc.sync.dma_start(out=bqc_tmp,
                          in_=bq_d[:].rearrange("(a p) -> p a", p=128))
        bq_col = consts.tile([128, NA], F32R, tag="bq_col")  # [a%128, a//128]
        nc.vector.tensor_copy(out=bq_col, in_=bqc_tmp)
        bk_col = consts.tile([128, NA], F32, tag="bk_col")   # ACT bias use only
        nc.sync.dma_start(out=bk_col,
                          in_=bk_d[:].rearrange("(a p) -> p a", p=128))

        # WqT[a, f] via 24 PE transposes (one-time); fp32r for W2 matmuls
        wqT_sb = consts.tile([128, NA, FD], F32R, tag="wqT")
        for ai in range(NA):
            for fj in range(NF):
                tp = psA.tile([128, 128], F32, tag="ft")
                nc.tensor.matmul(
                    tp, wq_tmp_v[:, fj, ai * 128:(ai + 1) * 128],
                    ident, is_transpose=True)
                nc.scalar.activation(
                    out=wqT_sb[:, ai, fj * 128:(fj + 1) * 128], in_=tp,
                    func=AF.Copy)

        for b in range(BPC):
            # ---- phoneme-side stage (per batch element) -----------------
            phn_sb = batchp.tile([128, NP, PD], F32, tag="phn")
            nc.sync.dma_start(
                out=phn_sb,
                in_=phn_d[b, :, :].rearrange("(s p) f -> p s f", p=128))

            phnT_sb = batchp.tile([128, NA, TP], F32R, tag="phnT")  # [f,fj,p]
            for fj in range(NA):
                pt = psD.tile([128, TP], F32, tag="at")
                for pi in range(NP):
                    nc.tensor.matmul(
                        pt[:, pi * 128:(pi + 1) * 128],
                        phn_sb[:, pi, fj * 128:(fj + 1) * 128], ident,
                        is_transpose=True,
                        start=(pi == 0), stop=(pi == NP - 1))
                nc.vector.tensor_copy(out=phnT_sb[:, fj, :], in_=pt)

            # kT[a, p] = sum_f Wk[f,a] phnT[f,p]; +bk along partitions
            kT_sb = batchp.tile([128, NA, TP], F32R, tag="kT")
            for ai in range(NA):
                kt = psE.tile([128, TP], F32, tag="o")
                for fj in range(NA):
                    nc.tensor.matmul(
                        kt, wk_sb[:, fj, ai * 128:(ai + 1) * 128],
                        phnT_sb[:, fj, :],
                        start=(fj == 0), stop=(fj == NA - 1))
                nc.scalar.activation(
                    out=kT_sb[:, ai, :], in_=kt, func=AF.Identity,
                    bias=bk_col[:, ai:ai + 1])

            # k[p, a] = sum_f phnT[f,p] Wk[f,a] + ones (x) bk
            k_sb = batchp.tile([128, NP, AD], F32R, tag="k")
            for pi in range(NP):
                kp = psB.tile([128, AD], F32, tag="q")
                for fj in range(NA):
                    nc.tensor.matmul(
                        kp, phnT_sb[:, fj, pi * 128:(pi + 1) * 128],
                        wk_sb[:, fj, :],
                        start=(fj == 0), stop=False)
                nc.tensor.matmul(kp, ones_col, bk_row, start=False, stop=True)
                nc.scalar.activation(out=k_sb[:, pi, :], in_=kp, func=AF.Copy)

            # W2[f, p] = sum_a Wq[f,a] kT[a,p]
            w2_sb = batchp.tile([128, NF, TP], F32R, tag="w2")
            for fj in range(NF):
                wp = psC.tile([128, TP], F32, tag="e")
                for ai in range(NA):
                    nc.tensor.matmul(
                        wp, wqT_sb[:, ai, fj * 128:(fj + 1) * 128],
                        kT_sb[:, ai, :],
                        start=(ai == 0), stop=(ai == NA - 1))
                nc.scalar.activation(out=w2_sb[:, fj, :], in_=wp, func=AF.Copy)

            # bias_row = bq @ kT + (mask-1)*1000
            mask_i = batchp.tile([1, TP], I32, tag="mask_i")
            nc.sync.dma_start(out=mask_i, in_=mask_d[b, :].unsqueeze(0))
            bias_f = batchp.tile([1, TP], F32, tag="bias_f")
            nc.vector.tensor_copy(out=bias_f, in_=mask_i)  # int -> float
            nc.vector.tensor_scalar(
                out=bias_f, in0=bias_f, scalar1=1000.0, scalar2=-1000.0,
                op0=ALU.mult, op1=ALU.add)
            e0 = psE.tile([1, TP], F32, tag="o")
            for ai in range(NA):
                nc.tensor.matmul(
                    e0, bq_col[:, ai:ai + 1], kT_sb[:, ai, :],
                    start=(ai == 0), stop=(ai == NA - 1))
            bias_row = batchp.tile([1, TP], F32R, tag="bias_row")
            nc.vector.tensor_add(out=bias_row, in0=bias_f, in1=e0)

            # ---- frame-side, per 512-row t-chunk ------------------------
            for ci in range(NTC):
                t0 = ci * 512
                frame_sb = chunkp.tile([128, 4 * FD], F32, tag="frame")
                frame_v = frame_sb.rearrange("p (s f) -> p s f", s=4)
                nc.sync.dma_start(
                    out=frame_v,
                    in_=frame_d[b, t0:t0 + 512, :].rearrange(
                        "(s p) f -> p s f", p=128))

                # frameT chunk: [f, fj, t(512)], rounded to fp32r on evict
                ft_sb = chunkp.tile([128, NF, 512], F32R, tag="ft")
                for fj in range(NF):
                    fp = psA.tile([128, 512], F32, tag="ft")
                    for ts in range(4):
                        nc.tensor.matmul(
                            fp[:, ts * 128:(ts + 1) * 128],
                            frame_v[:, ts, fj * 128:(fj + 1) * 128], ident,
                            is_transpose=True,
                            start=(ts == 0), stop=(ts == 3))
                    nc.scalar.activation(out=ft_sb[:, fj, :], in_=fp,
                                         func=AF.Copy)

                cat_sb = chunkp.tile([128, 4, 2 * AD], F32, tag="cat")
                energy_sb = chunkp.tile([128, 4, TP], F32, tag="energy")
                att_sb = chunkp.tile([128, 4, TP], F32, tag="att")

                for ts in range(4):
                    # q tile -> cat[:, ts, 512:1024]
                    qp = psB.tile([128, AD], F32, tag="q")
                    for fj in range(NF):
                        nc.tensor.matmul(
                            qp, ft_sb[:, fj, ts * 128:(ts + 1) * 128],
                            wq_sb[:, fj, :],
                            start=(fj == 0), stop=False)
                    nc.tensor.matmul(qp, ones_col, bq_row,
                                     start=False, stop=True)
                    nc.scalar.activation(out=cat_sb[:, ts, AD:2 * AD], in_=qp,
                                         func=AF.Copy)

                    # energy tile
                    ep = psC.tile([128, TP], F32, tag="e")
                    for fj in range(NF):
                        nc.tensor.matmul(
                            ep, ft_sb[:, fj, ts * 128:(ts + 1) * 128],
                            w2_sb[:, fj, :],
                            start=(fj == 0), stop=False)
                    nc.tensor.matmul(ep, ones_col, bias_row,
                                     start=False, stop=True)
                    nc.vector.tensor_copy(out=energy_sb[:, ts, :], in_=ep)
                    nc.scalar.activation(out=att_sb[:, ts, :], in_=ep,
                                         func=AF.Exp)

                nc.sync.dma_start(
                    out=energy_d[b, t0:t0 + 512, :].rearrange(
                        "(s p) f -> p s f", p=128),
                    in_=energy_sb)

                # softmax denominators
                s_sb = chunkp.tile([128, 4], F32, tag="s")
                nc.vector.reduce_sum(out=s_sb, in_=att_sb, axis=AX.X)
                r_sb = chunkp.tile([128, 4], F32, tag="r")
                nc.vector.reciprocal(out=r_sb, in_=s_sb)

                # attT: [p, pi, t(512)]
                at_sb = chunkp.tile([128, NP, 512], F32R, tag="atT")
                for pi in range(NP):
                    ap_ = psD.tile([128, 512], F32, tag="at")
                    for ts in range(4):
                        nc.tensor.matmul(
                            ap_[:, ts * 128:(ts + 1) * 128],
                            att_sb[:, ts, pi * 128:(pi + 1) * 128], ident,
                            is_transpose=True,
                            start=(ts == 0), stop=(ts == 3))
                    nc.vector.tensor_copy(out=at_sb[:, pi, :], in_=ap_)

                # att_out = (attU @ k) * r  -> cat[:, ts, 0:512]
                for ts in range(4):
                    op_ = psE.tile([128, AD], F32, tag="o")
                    for pi in range(NP):
                        nc.tensor.matmul(
                            op_, at_sb[:, pi, ts * 128:(ts + 1) * 128],
                            k_sb[:, pi, :],
                            start=(pi == 0), stop=(pi == NP - 1))
                    nc.scalar.activation(out=cat_sb[:, ts, 0:AD], in_=op_,
                                         func=AF.Copy,
                                         scale=r_sb[:, ts:ts + 1])

                # LayerNorm over 1024 (gamma/beta handled host-side)
                stats = chunkp.tile([128, 4, 2, 6], F32, tag="stats")
                mv = chunkp.tile([128, 4, 2], F32, tag="mv")
                for ts in range(4):
                    for g in range(2):
                        nc.vector.bn_stats(
                            out=stats[:, ts, g, :],
                            in_=cat_sb[:, ts, g * 512:(g + 1) * 512])
                    nc.vector.bn_aggr(out=mv[:, ts, :], in_=stats[:, ts, :, :])
                sd = chunkp.tile([128, 4], F32, tag="sd")
                nc.scalar.activation(out=sd, in_=mv[:, :, 1], func=AF.Sqrt,
                                     bias=eps_col)
                rstd = chunkp.tile([128, 4], F32, tag="rstd")
                nc.vector.reciprocal(out=rstd, in_=sd)
                for ts in range(4):
                    nc.vector.tensor_scalar(
                        out=cat_sb[:, ts, :], in0=cat_sb[:, ts, :],
                        scalar1=mv[:, ts, 0:1], scalar2=rstd[:, ts:ts + 1],
                        op0=ALU.subtract, op1=ALU.mult)

                nc.sync.dma_start(
                    out=attout_d[b, t0:t0 + 512, :].rearrange(
                        "(s p) f -> p s f", p=128),
                    in_=cat_sb)

    _split_excess_waits(nc)
    return nc


_NC_CACHE = []


def _get_program():
    if not _NC_CACHE:
        _NC_CACHE.append(build_program())
    return _NC_CACHE[0]


def make_in_maps(frame_hidden, phn_hidden, labels_att_mask, Wq, bq, Wk, bk):
    ins = []
    for c in range(N_CORES):
        s = slice(c * BPC, (c + 1) * BPC)
        ins.append({
            "frame": np.ascontiguousarray(frame_hidden[s], dtype=np.float32),
            "phn": np.ascontiguousarray(phn_hidden[s], dtype=np.float32),
            "mask": np.ascontiguousarray(labels_att_mask[s], dtype=np.int32),
            "wq": np.ascontiguousarray(Wq, dtype=np.float32),
            "bq": np.ascontiguousarray(bq, dtype=np.float32),
            "wk": np.ascontiguousarray(Wk, dtype=np.float32),
            "bk": np.ascontiguousarray(bk, dtype=np.float32),
        })
    return ins


def kernel(frame_hidden, phn_hidden, labels_att_mask, Wq, bq, Wk, bk,
           gamma, beta, _trace=False):
    nc = _get_program()
    in_maps = make_in_maps(frame_hidden, phn_hidden, labels_att_mask,
                           Wq, bq, Wk, bk)
    res = run_bass_kernel_spmd(nc, in_maps, core_ids=list(range(N_CORES)),
                               trace=_trace)
    att_out = np.concatenate(
        [res.results[c]["attout"] for c in range(N_CORES)], 0)
    energy = np.concatenate(
        [res.results[c]["energy"] for c in range(N_CORES)], 0)
    gamma = np.asarray(gamma, dtype=np.float32)
    beta = np.asarray(beta, dtype=np.float32)
    if not (np.all(gamma == 1.0) and np.all(beta == 0.0)):
        att_out = att_out * gamma + beta
    if _trace:
        return (att_out, energy), res
    return (att_out, energy)
